# revision 1
# baseline (speedup 1.0000x reference)
"""Trainium2 Bass kernel for the AdapterController hard-routing MoE adapter.

Reference computation (per router m in [0,4), batch b in [0,16)):
    e = expert_index[m, b]
    z = x[b] @ down_w[m, e] + down_b[m, e]      # [512, 256]
    z = z * sigmoid(z)                          # swish
    u = z @ up_w[m, e]                          # [512, 1024]
    out[m, b] = u

Strategy: data-parallel over the batch axis (2 batches per core, 8 cores).
The expert gather is part of input sharding: each core receives exactly the
(m, b)-selected weight matrices, packed on the host into the SBUF partition
layout so every DMA is fully contiguous.

On-chip per (m, b) pair:
    zT[d, s] = sum_c Wd[c, d] * xT[c, s]        (16 matmuls N=512, K=128)
    z = silu(zT + bd)                           (ACT engine, PSUM -> SBUF)
    u[s, c] = sum_d zT[d, s].T @ Wu[d, c]       (16 matmuls N=512)

Schedule notes: the matmul stream itself runs at the warm-PE roofline
(~216 ns per N=512 matmul), so the schedule optimizes the edges:
  - weights ride the scalar HWDGE ring, x + outputs ride the sync ring;
    each ring drains FIFO in trigger order, so emission order pins the
    byte-arrival order (first-pair inputs first, full-size transfers);
  - the bias transfer is padded to 576B rows (rows below the 512B SDMA
    line-rate minimum degrade into read-modify-write descriptors and clog
    the ring for ~2.5us);
  - a dense burst of dummy N=128 matmuls after a tiny memset lifts the PE
    HAM clock gate (1.2 -> 2.4 GHz) before the real matmuls arrive;
  - the first pair runs its down-projection k-outer so both PSUM groups
    chase each arriving x chunk;
  - the last output block is half-copied by vector+scalar in parallel and
    DMA'd from both rings to shorten the drain tail.
"""

import numpy as np

M, B, S, C, D = 4, 16, 512, 1024, 256
N_CORES = 8
B_LOC = B // N_CORES  # batches per core
KC = C // 128         # 8 c-chunks
KD = D // 128         # 2 d-chunks
NPAIR = M * B_LOC     # 8 (m, b) pairs per core

_cache = {}
last_results = None  # BassKernelResults of the most recent run (for test.py)


def _build():
    from contextlib import ExitStack

    import concourse.mybir as mybir
    import concourse.tile as tile
    from concourse import bacc
    f32 = mybir.dt.float32
    bf16 = mybir.dt.bfloat16
    mm_dt = bf16
    out_dt = bf16

    nc = bacc.Bacc("TRN2", target_bir_lowering=False, debug=False,
                   num_devices=N_CORES)
    # xtp[b, half][p, k*512 + s] = x[b, s, 128*(4*half + k) + p]
    xtp = nc.dram_tensor("xtp", [B_LOC, 2, 128, KC * S // 2], bf16,
                         kind="ExternalInput").ap()
    # wdp[m, b][p, k*256 + d] = down_w_gathered[m, b, 128k + p, d]
    wdp = nc.dram_tensor("wdp", [M, B_LOC, 128, KC * D], bf16,
                         kind="ExternalInput").ap()
    # bdp[p, (m*B_LOC+b)*2 + j] = down_b_gathered[m, b, 128j + p]; padded to
    # 144 f32 columns so each DMA row is 576B -- rows under the 512B SDMA
    # line-rate minimum (e.g. a bare [128,16] f32 = 64B/row) degrade the
    # whole ring to read-modify-write descriptors and clog it for ~2.5us
    bdp = nc.dram_tensor("bdp", [128, 144], f32, kind="ExternalInput").ap()
    # wup[m, b][p, j*1024 + c] = up_w_gathered[m, b, 128j + p, c]
    wup = nc.dram_tensor("wup", [M, B_LOC, 128, KD * C], bf16,
                         kind="ExternalInput").ap()
    out = nc.dram_tensor("out", [M, B_LOC, S, C], out_dt,
                         kind="ExternalOutput").ap()

    silu = mybir.ActivationFunctionType.Silu
    copy_fn = mybir.ActivationFunctionType.Copy

    with tile.TileContext(nc) as tc, ExitStack() as ctx:
        const = ctx.enter_context(tc.tile_pool(name="const", bufs=1))
        xpool = ctx.enter_context(tc.tile_pool(name="xpool", bufs=4))
        wdpool = ctx.enter_context(tc.tile_pool(name="wdpool", bufs=4))
        wupool = ctx.enter_context(tc.tile_pool(name="wupool", bufs=4))
        zpool = ctx.enter_context(tc.tile_pool(name="zpool", bufs=2))
        upool = ctx.enter_context(tc.tile_pool(name="upool", bufs=12))
        pszp = ctx.enter_context(tc.tile_pool(name="pszp", bufs=2, space="PSUM"))
        psup = ctx.enter_context(tc.tile_pool(name="psup", bufs=3, space="PSUM"))

        bd_sb = const.tile([128, 144], f32)

        # PE warm-up: the HAM clock gate needs ~3.4us of PE activity to
        # lift the 1.2GHz cold throttle; a tiny memset lets the burst start
        # ~1us earlier and N=128 keeps the burst dense
        warm_src = const.tile([128, 128], mm_dt)
        nc.gpsimd.memset(warm_src[:], 0)
        warm_ps = pszp.tile([128, 128], f32, tag="psz", name="warm_ps")
        for _ in range(36):
            nc.tensor.matmul(warm_ps[:], warm_src[:], warm_src[:],
                             start=True, stop=True)

        # Two HWDGE rings drain FIFO in trigger order, so emission order
        # pins byte-arrival order per ring. x + outputs ride the sync ring;
        # weights + bias ride the scalar ring. (No dep-chaining between
        # DMAs: a dep on a DMA instruction waits for its data semaphore,
        # which would serialize transfer-after-completion.)
        xh = {b: [xpool.tile([128, KC * S // 2], mm_dt, tag="xt",
                             name=f"xt_{b}_{h}") for h in range(2)]
              for b in range(B_LOC)}
        wd_t = {p: wdpool.tile([128, KC * D], mm_dt, tag="wd",
                               name=f"wd{p}")
                for p in range(2)}
        wu_t = {p: wupool.tile([128, KD * C], mm_dt, tag="wu", name=f"wu{p}")
                for p in range(2)}

        # Head fill in exact consumption order of the first two pairs
        # (pair p = b*M + m, so pairs 0/1 are (m=0/1, b=0)).
        nc.scalar.dma_start(wd_t[0][:], wdp[0, 0])   # wd0
        nc.sync.dma_start(xh[0][0][:], xtp[0, 0])    # x(b0) k0-3
        nc.sync.dma_start(xh[0][1][:], xtp[0, 1])    # x(b0) k4-7
        nc.scalar.dma_start(wd_t[1][:], wdp[1, 0])   # wd1
        nc.scalar.dma_start(bd_sb[:], bdp[:])        # bias (576B rows)
        nc.scalar.dma_start(wu_t[0][:], wup[0, 0])   # wu0
        nc.scalar.dma_start(wu_t[1][:], wup[1, 0])   # wu1

        for p in range(NPAIR):
            m, b = p % M, p // M
            if p == 1:
                nc.sync.dma_start(xh[1][0][:], xtp[1, 0])
                nc.sync.dma_start(xh[1][1][:], xtp[1, 1])
            q = p + 2
            if q < NPAIR:
                mq, bq = q % M, q // M
                wd_t[q] = wdpool.tile([128, KC * D], mm_dt, tag="wd",
                                      name=f"wd{q}")
                nc.scalar.dma_start(wd_t[q][:], wdp[mq, bq])
                wu_t[q] = wupool.tile([128, KD * C], mm_dt, tag="wu", name=f"wu{q}")
                nc.scalar.dma_start(wu_t[q][:], wup[mq, bq])

            wd_sb, wu_sb, xb = wd_t[p], wu_t[p], xh[b]
            z_sb = zpool.tile([128, KD, S], mm_dt)
            if p == 0:
                # k-outer for the first pair: both PSUM groups consume each
                # arriving x chunk, halving the early x-consumption rate so
                # the matmuls bridge the x(b0) half-1 delivery
                psz_j = [pszp.tile([128, S], f32, tag="psz", name=f"psz{j}")
                         for j in range(KD)]
                for k in range(KC):
                    for j in range(KD):
                        nc.tensor.matmul(
                            psz_j[j][:],
                            wd_sb[:, k * 256 + j * 128:
                                  k * 256 + j * 128 + 128],
                            xb[k // 4][:, (k % 4) * S: (k % 4 + 1) * S],
                            start=(k == 0), stop=(k == KC - 1),
                        )
                for j in range(KD):
                    col = (m * B_LOC + b) * KD + j
                    nc.scalar.activation(z_sb[:, j, :], psz_j[j][:], silu,
                                         bias=bd_sb[:, col: col + 1])
            else:
                for j in range(KD):
                    psz = pszp.tile([128, S], f32)
                    for k in range(KC):
                        nc.tensor.matmul(
                            psz[:],
                            wd_sb[:, k * 256 + j * 128:
                                  k * 256 + j * 128 + 128],
                            xb[k // 4][:, (k % 4) * S: (k % 4 + 1) * S],
                            start=(k == 0), stop=(k == KC - 1),
                        )
                    col = (m * B_LOC + b) * KD + j
                    nc.scalar.activation(z_sb[:, j, :], psz[:], silu,
                                         bias=bd_sb[:, col: col + 1])

            # one 2-bank PSUM tile per a holds a full [128, 1024] u row;
            # groups are interleaved so j=1 matmuls trail the j=1 silu by a
            # couple of matmul slots (no PE stall). The last pair finishes
            # a2 earlier so the vector engine is free to take half of a3's
            # final copy in parallel with the scalar engine.
            last = p == NPAIR - 1
            if last:
                order = ((0, 0), (0, 1), (1, 0), (1, 1),
                         (0, 2), (1, 2), (0, 3), (1, 3))
            else:
                order = ((0, 0), (0, 1), (1, 0), (1, 1),
                         (0, 2), (0, 3), (1, 2), (1, 3))
            psu_by_a = {}
            for j, a in order:
                if j == 0:
                    psu_by_a[a] = psup.tile([128, C], f32, tag="psu",
                                            name=f"psu_{p}_{a}")
                psu = psu_by_a[a]
                for h in range(2):
                    nc.tensor.matmul(
                        psu[:, h * 512: (h + 1) * 512],
                        z_sb[:, j, a * 128: (a + 1) * 128],
                        wu_sb[:, j * 1024 + h * 512:
                              j * 1024 + h * 512 + 512],
                        start=(j == 0), stop=(j == KD - 1),
                        skip_group_check=True,
                    )
                if j == KD - 1:
                    u_sb = upool.tile([128, C], out_dt, tag="u")
                    orow = out[m, b, a * 128:(a + 1) * 128, :]
                    if last and a == 3:
                        # final block: halves copied by both engines in
                        # parallel (different PSUM banks), each half DMA'd
                        # from its own ring the moment its copy lands
                        nc.scalar.activation(u_sb[:, 512:], psu[:, 512:],
                                             copy_fn)
                        nc.scalar.dma_start(orow[:, 512:], u_sb[:, 512:])
                        nc.vector.tensor_copy(u_sb[:, :512], psu[:, :512])
                        nc.sync.dma_start(orow[:, :512], u_sb[:, :512])
                    else:
                        if a % 2 == 0:
                            nc.vector.tensor_copy(u_sb[:], psu[:])
                        else:
                            nc.scalar.activation(u_sb[:], psu[:], copy_fn)
                        nc.sync.dma_start(orow, u_sb[:])

    nc.compile()
    return nc


def _get_nc():
    if "nc" not in _cache:
        _cache["nc"] = _build()
    return _cache["nc"]


def kernel(x, expert_index, down_w, down_b, up_w):
    global last_results
    import ml_dtypes
    from concourse import bass_utils

    x = np.asarray(x, dtype=np.float32)
    idx = np.asarray(expert_index)
    r = np.arange(M)[:, None]
    wd = np.asarray(down_w, dtype=np.float32)[r, idx]   # [M, B, C, D]
    bd = np.asarray(down_b, dtype=np.float32)[r, idx]   # [M, B, D]
    wu = np.asarray(up_w, dtype=np.float32)[r, idx]     # [M, B, D, C]

    # Pack into SBUF partition-major layouts (see _build comments).
    xt = x.transpose(0, 2, 1).reshape(B, 2, KC // 2, 128, S)
    xt = xt.transpose(0, 1, 3, 2, 4).reshape(B, 2, 128, KC * S // 2)
    wdp = wd.reshape(M, B, KC, 128, D).transpose(0, 1, 3, 2, 4)
    wdp = wdp.reshape(M, B, 128, KC * D)
    wup = wu.reshape(M, B, KD, 128, C).transpose(0, 1, 3, 2, 4)
    wup = wup.reshape(M, B, 128, KD * C)
    bdp = bd.reshape(M, B, KD, 128).transpose(3, 0, 1, 2)  # [128, M, B, KD]

    in_dt = ml_dtypes.bfloat16

    in_maps = []
    for i in range(N_CORES):
        bs = slice(i * B_LOC, (i + 1) * B_LOC)
        # bias rows padded to 576B (see _build): cols 0:16 real, rest zero
        bias_pad = np.zeros((128, 144), dtype=np.float32)
        bias_pad[:, :M * B_LOC * KD] = \
            bdp[:, :, bs, :].reshape(128, M * B_LOC * KD)
        in_maps.append({
            "xtp": np.ascontiguousarray(xt[bs].astype(in_dt)),
            "wdp": np.ascontiguousarray(wdp[:, bs].astype(in_dt)),
            "wup": np.ascontiguousarray(wup[:, bs].astype(in_dt)),
            "bdp": bias_pad,
        })

    nc = _get_nc()
    res = None
    for attempt in range(3):
        try:
            res = bass_utils.run_bass_kernel_spmd(nc, in_maps,
                                                  core_ids=list(range(N_CORES)))
            break
        except Exception:
            # transient NRT_EXEC_UNIT_UNRECOVERABLE device hiccups recover
            # after a short wait; re-raise if persistent
            if attempt == 2:
                raise
            import time
            time.sleep(15)
    last_results = res

    full = np.empty((M, B, S, C), dtype=np.float32)
    for i in range(N_CORES):
        full[:, i * B_LOC:(i + 1) * B_LOC] = np.asarray(
            res.results[i]["out"]).astype(np.float32)
    return full

